# revision 10
# baseline (speedup 1.0000x reference)
"""Masked multi-head self-attention on 8 Trainium2 NeuronCores.

Problem: B=4, T=1024, C=1024, H=16 heads (D=64), key-padding mask.
Sharding: core c handles batch b=c//2 and heads [8*(c%2), 8*(c%2)+8)
(data parallel on B x tensor parallel on heads). Each core computes its
partial output projection; host sums the two head-half partials per batch
and adds bp.

Since mask = keep[q] * keep[k] with the same keep flag on both sides, the
host compacts away padded rows entirely: only the TT = roundup(max kept)
rows participate on-device (queries AND keys shrink ~20%). Padded-query
outputs are reconstructed exactly on the host (reference gives them
uniform attention over all keys).

Per-core device algorithm (transposed layouts, contraction on partitions):
  QT = Wq_c^T xg^T  [512, TT]   KT = Wk_c^T xg^T  [512, TT]
  V_aug[kt,h]: even h -> v in cols 0:64, ones in cols 64:128
               odd  h -> ones in cols 0:64, v in cols 64:128
  S^T_h = KT_h^T QT_h * 1/8 (+ -1e9 tail-key bias) -> exp (bf16)
  yps_h = V_aug_h^T expS_h : y rows on one half, row-sums broadcast on the
          other half of the 128 PSUM partitions.
  bpsf = 1/sums via DVE reciprocal, y = yps * bpsf (normalize fused into
  PSUM eviction), out = y^T Wp accumulated over head pairs.
"""

import sys

sys.path.insert(0, "/opt/trn_rl_repo")

import math

import ml_dtypes
import numpy as np

import concourse.bass as bass
import concourse.tile as tile
from concourse import mybir
from concourse.bass_utils import run_bass_kernel_spmd

B, T, C, H = 4, 1024, 1024, 16
D = C // H          # 64 head dim
HL = H // 2         # 8 heads per core
CP = HL * D         # 512 per-core projection width
P = 128
KT = C // P         # 8 contraction subtiles of C
MT = CP // P        # 4 m-tiles of QT/KT
BF16 = mybir.dt.bfloat16
F32 = mybir.dt.float32

LAST_RESULTS = None  # BassKernelResults of the most recent run (for test.py)


# ---------------------------------------------------------------------------
# Workaround: this walrus build only accepts ONE sync-wait command per
# instruction, but Tile's sem assignment can attach several. Post-pass: move
# extra waits onto fresh same-engine nops inserted just before the carrier.
def _split_multi_waits(nc):
    n = 0
    for f in nc.m.functions:
        for blk in f.blocks:
            newlist, changed = [], False
            for i in blk.instructions:
                si = i.sync_info
                if si is not None and si.on_wait is not None and len(si.on_wait) > 1:
                    w = list(si.on_wait)
                    for ww in w[:-1]:
                        newlist.append(
                            mybir.InstNoOp(
                                name=f"WSPLIT-{n}",
                                engine=i.engine,
                                sync_info=mybir.SyncInfo(on_wait=[ww], on_update=[]),
                            )
                        )
                        n += 1
                    si.on_wait = [w[-1]]
                    changed = True
                newlist.append(i)
            if changed:
                blk.instructions = newlist


# NTFF profiling hook: bass_utils' axon trace path looks for
# antenv.axon_hooks, which this image lacks. Synthesize it and register the
# ctypes-based profiler from trn_agent_boot so BASS_TRACE=1 yields exec times.
def _register_ntff_hook():
    try:
        import antenv.axon_hooks  # noqa: F401
        return
    except ImportError:
        pass
    try:
        import types

        import antenv
        from trn_agent_boot.trn_boot import _ntff_profile_via_ctypes

        mod = types.ModuleType("antenv.axon_hooks")
        _state = {"hook": None}
        mod.set_axon_ntff_profile_hook = lambda h: _state.__setitem__("hook", h)
        mod.get_axon_ntff_profile_hook = lambda: _state["hook"]
        sys.modules["antenv.axon_hooks"] = mod
        antenv.axon_hooks = mod
        so = "/opt/axon/libaxon_pjrt.so"
        import os

        if os.path.exists(so):
            mod.set_axon_ntff_profile_hook(_ntff_profile_via_ctypes(so))
    except Exception:
        pass


_register_ntff_hook()
# ---------------------------------------------------------------------------


def _build_nc(TT):
    NT = TT // P
    CHS = [(0, 512), (512, TT)] if TT > 512 else [(0, TT)]
    nc = bass.Bass()
    x = nc.dram_tensor("xh", [P, KT * TT], BF16, kind="ExternalInput")
    wq = nc.dram_tensor("wq", [P, KT * CP], BF16, kind="ExternalInput")
    wk = nc.dram_tensor("wk", [P, KT * CP], BF16, kind="ExternalInput")
    wv = nc.dram_tensor("wv", [P, KT * CP], BF16, kind="ExternalInput")
    wp = nc.dram_tensor("wp", [P, MT * C], BF16, kind="ExternalInput")
    bq = nc.dram_tensor("bq", [P, MT], F32, kind="ExternalInput")
    bk = nc.dram_tensor("bk", [P, MT], F32, kind="ExternalInput")
    bv = nc.dram_tensor("bv", [P, CP], F32, kind="ExternalInput")
    mk = nc.dram_tensor("mk", [P, NT], F32, kind="ExternalInput")
    out = nc.dram_tensor("out", [TT, C], BF16, kind="ExternalOutput")

    EXP = mybir.ActivationFunctionType.Exp
    LN = mybir.ActivationFunctionType.Ln
    SCALE = 1.0 / math.sqrt(D)

    with tile.TileContext(nc) as tc:
        with (
            tc.tile_pool(name="consts", bufs=1) as consts,
            tc.tile_pool(name="expp", bufs=16) as expp,
            tc.tile_pool(name="outp", bufs=3) as outp,
            tc.tile_pool(name="ps2", bufs=2, space="PSUM") as ps2,
            tc.tile_pool(name="psy", bufs=2, space="PSUM") as psy,
            tc.tile_pool(name="ps1", bufs=2, space="PSUM") as ps1,
        ):
            # ---- input DMAs: priority-ordered, chunked ----------------------
            # sync queue: x by kt (first matmul needs x kt0 + wq kt0 only)
            x_sb = consts.tile([P, KT, TT], BF16)
            for kt in range(KT):
                nc.sync.dma_start(x_sb[:, kt, :], x[:, kt * TT : (kt + 1) * TT])
            # scalar queue: wq chunks, then wk chunks, then wv, wp
            wq_sb = consts.tile([P, KT, CP], BF16)
            wk_sb = consts.tile([P, KT, CP], BF16)
            for kt in range(KT):
                nc.scalar.dma_start(wq_sb[:, kt, :], wq[:, kt * CP : (kt + 1) * CP])
            for kt in range(KT):
                nc.scalar.dma_start(wk_sb[:, kt, :], wk[:, kt * CP : (kt + 1) * CP])
            wv_sb = consts.tile([P, KT, CP], BF16)
            nc.scalar.dma_start(wv_sb[:], wv.rearrange("p (kt n) -> p kt n", kt=KT))
            wp_sb = consts.tile([P, MT, C], BF16)
            nc.scalar.dma_start(wp_sb[:], wp.rearrange("p (s n) -> p s n", s=MT))
            # vector queue: small consts
            bq_sb = consts.tile([P, MT], F32)
            nc.gpsimd.dma_start(bq_sb[:], bq[:])
            bk_sb = consts.tile([P, MT], F32)
            nc.gpsimd.dma_start(bk_sb[:], bk[:])
            bv_sb = consts.tile([P, CP], F32)
            nc.gpsimd.dma_start(bv_sb[:], bv[:])
            mk_sb = consts.tile([P, NT], F32)
            nc.gpsimd.dma_start(mk_sb[:], mk[:])

            # ---- persistent SBUF tensors ------------------------------------
            # V_aug layout [p, kt, h, m]: even h -> v at m 0:64, ones 64:128;
            # odd h -> ones at 0:64, v at 64:128.
            V_sb = consts.tile([P, NT, HL, P], BF16)
            QT_sb = consts.tile([P, MT, TT], BF16)
            KT_sb = consts.tile([P, MT, TT], BF16)
            y_sb = consts.tile([P, MT, TT], BF16)
            bpsf = consts.tile([P, TT], BF16)
            lnst = consts.tile([P, TT], F32)

            V5 = V_sb.rearrange("p kt (hh par) m -> p kt hh par m", par=2)
            nh = NT // 2
            nc.gpsimd.memset(V5[:, 0:nh, :, 0, 0:D], 0.0)
            nc.gpsimd.memset(V5[:, nh:NT, :, 0, 0:D], 0.0)
            nc.vector.memset(V5[:, 0:nh, :, 1, D:P], 0.0)
            nc.vector.memset(V5[:, nh:NT, :, 1, D:P], 0.0)
            nc.gpsimd.memset(V5[:, :, :, 0, D:P], 1.0)
            nc.gpsimd.memset(V5[:, :, :, 1, 0:D], 1.0)

            # ---- emission helpers -------------------------------------------
            def qk_proj_steps(mt):
                """Q & K projection of m-tile mt, as filler closures."""
                steps = []
                for w_sb, b_sb, dst in ((wq_sb, bq_sb, QT_sb), (wk_sb, bk_sb, KT_sb)):
                    for c0, c1 in CHS:
                        def step(w_sb=w_sb, b_sb=b_sb, dst=dst, c0=c0, c1=c1):
                            ps = ps1.tile([P, c1 - c0], F32, tag="ps1")
                            for kt in range(KT):
                                nc.tensor.matmul(
                                    ps[:],
                                    w_sb[:, kt, mt * P : (mt + 1) * P],
                                    x_sb[:, kt, c0:c1],
                                    start=(kt == 0),
                                    stop=(kt == KT - 1),
                                )
                            nc.vector.tensor_scalar_add(
                                dst[:, mt, c0:c1], ps[:], b_sb[:, mt : mt + 1]
                            )
                        steps.append(step)
                return steps

            def v_proj_steps():
                steps = []
                for tt in range(NT):
                    def step(tt=tt):
                        psv = ps1.tile([P, CP], F32, tag="ps1")
                        for kt in range(KT):
                            nc.tensor.matmul(
                                psv[:],
                                x_sb[:, kt, tt * P : (tt + 1) * P],
                                wv_sb[:, kt, :],
                                start=(kt == 0),
                                stop=(kt == KT - 1),
                            )
                        src = psv.rearrange("p (hh par d) -> p par hh d", par=2, d=D)
                        bvr = bv_sb.rearrange("p (hh par d) -> p par hh d", par=2, d=D)
                        nc.vector.tensor_add(V5[:, tt, :, 0, 0:D], src[:, 0], bvr[:, 0])
                        nc.vector.tensor_add(V5[:, tt, :, 1, D:P], src[:, 1], bvr[:, 1])
                    steps.append(step)
                return steps

            def out_proj_steps(s2lo, s2hi, mode):
                """Output projection contribution of head-pairs [s2lo, s2hi).

                mode: "init" psum -> oacc copy; "accum" psum + oacc -> oacc;
                "final" psum + oacc -> bf16 out tile, DMA (whole rows, queues
                round-robined to spread descriptor load).
                """
                dmaq = [nc.sync, nc.scalar, nc.gpsimd]
                steps = []
                for tt in range(NT):
                    ots = [None]
                    for ci, (c0, c1) in enumerate(((0, 512), (512, C))):
                        def step(tt=tt, ci=ci, c0=c0, c1=c1, ots=ots):
                            ps = ps1.tile([P, c1 - c0], F32, tag="ps1")
                            for s2 in range(s2lo, s2hi):
                                nc.tensor.matmul(
                                    ps[:],
                                    y_sb[:, s2, tt * P : (tt + 1) * P],
                                    wp_sb[:, s2, c0:c1],
                                    start=(s2 == s2lo),
                                    stop=(s2 == s2hi - 1),
                                )
                            if mode == "init":
                                nc.vector.tensor_copy(oacc[:, tt, c0:c1], ps[:])
                            elif mode == "accum":
                                nc.vector.tensor_add(
                                    oacc[:, tt, c0:c1], ps[:], oacc[:, tt, c0:c1]
                                )
                            else:
                                if ots[0] is None:
                                    ots[0] = outp.tile(
                                        [P, C], BF16, tag="out", name=f"ot{tt}"
                                    )
                                ot = ots[0]
                                nc.vector.tensor_add(
                                    ot[:, c0:c1], ps[:], oacc[:, tt, c0:c1]
                                )
                                if ci == 1:
                                    dmaq[tt % 3].dma_start(
                                        out[tt * P : (tt + 1) * P, :], ot[:]
                                    )
                        steps.append(step)
                return steps

            oacc = consts.tile([P, NT, C], F32)

            def attn_pair(p, fillers):
                """Attention for head pair (2p, 2p+1); pops filler closures
                between tensor bursts to keep the PE busy while Scalar exp
                and DVE normalize catch up."""
                def fill(n):
                    for _ in range(n):
                        if fillers:
                            fillers.pop(0)()

                hA, hB = 2 * p, 2 * p + 1
                eAs, eBs = [], []
                for kt in range(NT):
                    psA = ps2.tile([P, TT], F32, tag="ps2")
                    psB = ps2.tile([P, TT], F32, tag="ps2")
                    for c0, c1 in CHS:
                        nc.tensor.matmul(
                            psA[:, c0:c1],
                            KT_sb[0:D, p, kt * P : (kt + 1) * P],
                            QT_sb[0:D, p, c0:c1],
                            start=True,
                            stop=True,
                        )
                        nc.tensor.matmul(
                            psB[:, c0:c1],
                            KT_sb[D:P, p, kt * P : (kt + 1) * P],
                            QT_sb[D:P, p, c0:c1],
                            start=True,
                            stop=True,
                        )
                    eA = expp.tile([P, TT], BF16, tag="exp")
                    nc.scalar.activation(
                        eA[:], psA[:], EXP, bias=mk_sb[:, kt : kt + 1], scale=SCALE
                    )
                    eB = expp.tile([P, TT], BF16, tag="exp")
                    nc.scalar.activation(
                        eB[:], psB[:], EXP, bias=mk_sb[:, kt : kt + 1], scale=SCALE
                    )
                    eAs.append(eA)
                    eBs.append(eB)
                    fill(1)

                for h, es in ((hA, eAs), (hB, eBs)):
                    yr = slice(0, D) if h % 2 == 0 else slice(D, P)
                    sr = slice(D, P) if h % 2 == 0 else slice(0, D)
                    yps = [
                        psy.tile([P, c1 - c0], F32, tag="psy", name=f"yps{ci}")
                        for ci, (c0, c1) in enumerate(CHS)
                    ]
                    for kt in range(NT):
                        for ci, (c0, c1) in enumerate(CHS):
                            nc.tensor.matmul(
                                yps[ci][:],
                                V_sb[:, kt, h, :],
                                es[kt][:, c0:c1],
                                start=(kt == 0),
                                stop=(kt == NT - 1),
                            )
                    # normalize: 1/rowsums = exp(-ln(sums)) on Scalar (Ln and
                    # Exp share one act table), fused multiply on DVE eviction
                    for ci, (c0, c1) in enumerate(CHS):
                        nc.scalar.activation(
                            lnst[sr, c0:c1], yps[ci][sr, :], LN
                        )
                    nc.scalar.activation(bpsf[sr, :], lnst[sr, :], EXP, scale=-1.0)
                    for ci, (c0, c1) in enumerate(CHS):
                        nc.vector.tensor_mul(
                            y_sb[yr, p, c0:c1], yps[ci][yr, :], bpsf[sr, c0:c1]
                        )
                    fill(2)

            fillers0 = v_proj_steps() + qk_proj_steps(1)
            fillers1 = qk_proj_steps(2)
            fillers2 = qk_proj_steps(3) + out_proj_steps(0, 2, mode="init")
            fillers3 = out_proj_steps(2, 3, mode="accum")

            for step in qk_proj_steps(0):
                step()
            attn_pair(0, fillers0)
            for f in fillers0:
                f()
            attn_pair(1, fillers1)
            for f in fillers1:
                f()
            attn_pair(2, fillers2)
            for f in fillers2:
                f()
            attn_pair(3, fillers3)
            for f in fillers3:
                f()
            for step in out_proj_steps(3, 4, mode="final"):
                step()

    _split_multi_waits(nc)
    return nc


_NC_CACHE = {}


def _get_nc(TT):
    if TT not in _NC_CACHE:
        _NC_CACHE[TT] = _build_nc(TT)
    return _NC_CACHE[TT]


def kernel(x, x_padding_judge, Wq, bq, Wk, bk, Wv, bv, Wp, bp):
    global LAST_RESULTS
    x = np.asarray(x, dtype=np.float32)
    pad = np.asarray(x_padding_judge, dtype=np.float32)
    Wq = np.asarray(Wq, dtype=np.float32)
    Wk = np.asarray(Wk, dtype=np.float32)
    Wv = np.asarray(Wv, dtype=np.float32)
    Wp = np.asarray(Wp, dtype=np.float32)
    bq = np.asarray(bq, dtype=np.float32)
    bk = np.asarray(bk, dtype=np.float32)
    bv = np.asarray(bv, dtype=np.float32)
    bp = np.asarray(bp, dtype=np.float32)
    bf = ml_dtypes.bfloat16

    idxs = [np.nonzero(pad[b] == 0.0)[0] for b in range(B)]
    tns = [len(ix) for ix in idxs]
    TT = max(P, -(-max(tns) // P) * P)

    def pmajor(m, kt):  # [kt*P, n] -> [P, kt*n]
        return np.ascontiguousarray(
            m.reshape(kt, P, -1).transpose(1, 0, 2).reshape(P, -1)
        )

    in_maps = []
    for c in range(8):
        b, s = c // 2, c % 2
        cols = slice(s * CP, (s + 1) * CP)
        xg = np.zeros((TT, C), dtype=np.float32)
        xg[: tns[b]] = x[b][idxs[b]]
        mkv = np.full(TT, -1e9, dtype=np.float32)
        mkv[: tns[b]] = 0.0
        in_maps.append({
            "xh": pmajor(np.ascontiguousarray(xg.T), KT).astype(bf),
            "wq": pmajor(Wq[:, cols], KT).astype(bf),
            "wk": pmajor(Wk[:, cols], KT).astype(bf),
            "wv": pmajor(Wv[:, cols], KT).astype(bf),
            "wp": pmajor(Wp[cols, :], MT).astype(bf),
            "bq": np.ascontiguousarray(bq[cols].reshape(MT, P).T),
            "bk": np.ascontiguousarray(bk[cols].reshape(MT, P).T),
            "bv": np.broadcast_to(bv[cols], (P, CP)).copy(),
            "mk": np.ascontiguousarray(mkv.reshape(TT // P, P).T),
        })

    res = run_bass_kernel_spmd(_get_nc(TT), in_maps, core_ids=list(range(8)))
    LAST_RESULTS = res

    out = np.empty((B, T, C), dtype=np.float32)
    for b in range(B):
        part = res.results[2 * b]["out"].astype(np.float32) + res.results[
            2 * b + 1
        ]["out"].astype(np.float32)
        out[b, idxs[b], :] = part[: tns[b]] + bp

    # fully-padded query rows: uniform attention over ALL keys
    for b in range(B):
        rows = np.nonzero(pad[b] == 1.0)[0]
        if rows.size:
            xbar = x[b].mean(axis=0)
            out[b, rows, :] = (xbar @ Wv + bv) @ Wp + bp
    return out


# revision 11
# speedup vs baseline: 1.1182x; 1.1182x over previous
"""Masked multi-head self-attention on 8 Trainium2 NeuronCores.

Problem: B=4, T=1024, C=1024, H=16 heads (D=64), key-padding mask.
Sharding: core c handles batch b=c//2 and heads [8*(c%2), 8*(c%2)+8)
(data parallel on B x tensor parallel on heads). Each core computes its
partial output projection; host sums the two head-half partials per batch
and adds bp.

Since mask = keep[q] * keep[k] with the same keep flag on both sides, the
host compacts away padded rows entirely: only the TT = roundup(max kept)
rows participate on-device (queries AND keys shrink ~20%). Padded-query
outputs are reconstructed exactly on the host (reference gives them
uniform attention over all keys).

Per-core device algorithm (transposed layouts, contraction on partitions):
  QT = Wq_c^T xg^T  [512, TT]   KT = Wk_c^T xg^T  [512, TT]
  V_aug[kt,h]: even h -> v in cols 0:64, ones in cols 64:128
               odd  h -> ones in cols 0:64, v in cols 64:128
  S^T_h = KT_h^T QT_h * 1/8 (+ -1e9 tail-key bias) -> exp (bf16)
  yps_h = V_aug_h^T expS_h : y rows on one half, row-sums broadcast on the
          other half of the 128 PSUM partitions.
  bpsf = 1/sums via DVE reciprocal, y = yps * bpsf (normalize fused into
  PSUM eviction), out = y^T Wp accumulated over head pairs.
"""

import sys

sys.path.insert(0, "/opt/trn_rl_repo")

import math

import ml_dtypes
import numpy as np

import concourse.bass as bass
import concourse.tile as tile
from concourse import mybir
from concourse.bass_utils import run_bass_kernel_spmd

B, T, C, H = 4, 1024, 1024, 16
D = C // H          # 64 head dim
HL = H // 2         # 8 heads per core
CP = HL * D         # 512 per-core projection width
P = 128
KT = C // P         # 8 contraction subtiles of C
MT = CP // P        # 4 m-tiles of QT/KT
BF16 = mybir.dt.bfloat16
F32 = mybir.dt.float32

LAST_RESULTS = None  # BassKernelResults of the most recent run (for test.py)


# ---------------------------------------------------------------------------
# Workaround: this walrus build only accepts ONE sync-wait command per
# instruction, but Tile's sem assignment can attach several. Post-pass: move
# extra waits onto fresh same-engine nops inserted just before the carrier.
def _split_multi_waits(nc):
    n = 0
    for f in nc.m.functions:
        for blk in f.blocks:
            newlist, changed = [], False
            for i in blk.instructions:
                si = i.sync_info
                if si is not None and si.on_wait is not None and len(si.on_wait) > 1:
                    w = list(si.on_wait)
                    for ww in w[:-1]:
                        newlist.append(
                            mybir.InstNoOp(
                                name=f"WSPLIT-{n}",
                                engine=i.engine,
                                sync_info=mybir.SyncInfo(on_wait=[ww], on_update=[]),
                            )
                        )
                        n += 1
                    si.on_wait = [w[-1]]
                    changed = True
                newlist.append(i)
            if changed:
                blk.instructions = newlist


# NTFF profiling hook: bass_utils' axon trace path looks for
# antenv.axon_hooks, which this image lacks. Synthesize it and register the
# ctypes-based profiler from trn_agent_boot so BASS_TRACE=1 yields exec times.
def _register_ntff_hook():
    try:
        import antenv.axon_hooks  # noqa: F401
        return
    except ImportError:
        pass
    try:
        import types

        import antenv
        from trn_agent_boot.trn_boot import _ntff_profile_via_ctypes

        mod = types.ModuleType("antenv.axon_hooks")
        _state = {"hook": None}
        mod.set_axon_ntff_profile_hook = lambda h: _state.__setitem__("hook", h)
        mod.get_axon_ntff_profile_hook = lambda: _state["hook"]
        sys.modules["antenv.axon_hooks"] = mod
        antenv.axon_hooks = mod
        so = "/opt/axon/libaxon_pjrt.so"
        import os

        if os.path.exists(so):
            mod.set_axon_ntff_profile_hook(_ntff_profile_via_ctypes(so))
    except Exception:
        pass


_register_ntff_hook()
# ---------------------------------------------------------------------------


def _build_nc(TT):
    NT = TT // P
    CHS = [(0, 512), (512, TT)] if TT > 512 else [(0, TT)]
    nc = bass.Bass()
    x = nc.dram_tensor("xh", [P, KT * TT], BF16, kind="ExternalInput")
    wq = nc.dram_tensor("wq", [P, KT * CP], BF16, kind="ExternalInput")
    wk = nc.dram_tensor("wk", [P, KT * CP], BF16, kind="ExternalInput")
    wv = nc.dram_tensor("wv", [P, KT * CP], BF16, kind="ExternalInput")
    wp = nc.dram_tensor("wp", [P, MT * C], BF16, kind="ExternalInput")
    bq = nc.dram_tensor("bq", [P, MT], F32, kind="ExternalInput")
    bk = nc.dram_tensor("bk", [P, MT], F32, kind="ExternalInput")
    bv = nc.dram_tensor("bv", [P, CP], F32, kind="ExternalInput")
    mk = nc.dram_tensor("mk", [P, NT], F32, kind="ExternalInput")
    out = nc.dram_tensor("out", [TT, C], BF16, kind="ExternalOutput")

    EXP = mybir.ActivationFunctionType.Exp
    LN = mybir.ActivationFunctionType.Ln
    SCALE = 1.0 / math.sqrt(D)

    with tile.TileContext(nc) as tc:
        with (
            tc.tile_pool(name="consts", bufs=1) as consts,
            tc.tile_pool(name="expp", bufs=16) as expp,
            tc.tile_pool(name="outp", bufs=3) as outp,
            tc.tile_pool(name="ps2", bufs=2, space="PSUM") as ps2,
            tc.tile_pool(name="psy", bufs=2, space="PSUM") as psy,
            tc.tile_pool(name="ps1", bufs=2, space="PSUM") as ps1,
        ):
            # ---- input DMAs: priority-ordered, chunked ----------------------
            # sync queue: x by kt (first matmul needs x kt0 + wq kt0 only)
            x_sb = consts.tile([P, KT, TT], BF16)
            for kt in range(KT):
                nc.sync.dma_start(x_sb[:, kt, :], x[:, kt * TT : (kt + 1) * TT])
            # scalar queue: wq chunks, then wk chunks, then wv, wp
            wq_sb = consts.tile([P, KT, CP], BF16)
            wk_sb = consts.tile([P, KT, CP], BF16)
            for kt in range(KT):
                nc.scalar.dma_start(wq_sb[:, kt, :], wq[:, kt * CP : (kt + 1) * CP])
            for kt in range(KT):
                nc.scalar.dma_start(wk_sb[:, kt, :], wk[:, kt * CP : (kt + 1) * CP])
            wv_sb = consts.tile([P, KT, CP], BF16)
            nc.scalar.dma_start(wv_sb[:], wv.rearrange("p (kt n) -> p kt n", kt=KT))
            wp_sb = consts.tile([P, MT, C], BF16)
            nc.scalar.dma_start(wp_sb[:], wp.rearrange("p (s n) -> p s n", s=MT))
            # vector queue: small consts
            bq_sb = consts.tile([P, MT], F32)
            nc.gpsimd.dma_start(bq_sb[:], bq[:])
            bk_sb = consts.tile([P, MT], F32)
            nc.gpsimd.dma_start(bk_sb[:], bk[:])
            bv_sb = consts.tile([P, CP], F32)
            nc.gpsimd.dma_start(bv_sb[:], bv[:])
            mk_sb = consts.tile([P, NT], F32)
            nc.gpsimd.dma_start(mk_sb[:], mk[:])

            # ---- persistent SBUF tensors ------------------------------------
            # V_aug layout [p, kt, h, m]: even h -> v at m 0:64, ones 64:128;
            # odd h -> ones at 0:64, v at 64:128.
            V_sb = consts.tile([P, NT, HL, P], BF16)
            QT_sb = consts.tile([P, MT, TT], BF16)
            KT_sb = consts.tile([P, MT, TT], BF16)
            y_sb = consts.tile([P, MT, TT], BF16)
            bpsf = consts.tile([P, TT], BF16)
            lnst = consts.tile([P, TT], F32)

            V5 = V_sb.rearrange("p kt (hh par) m -> p kt hh par m", par=2)
            nh = NT // 2
            nc.gpsimd.memset(V5[:, 0:nh, :, 0, 0:D], 0.0)
            nc.gpsimd.memset(V5[:, nh:NT, :, 0, 0:D], 0.0)
            nc.vector.memset(V5[:, 0:nh, :, 1, D:P], 0.0)
            nc.vector.memset(V5[:, nh:NT, :, 1, D:P], 0.0)
            nc.gpsimd.memset(V5[:, :, :, 0, D:P], 1.0)
            nc.gpsimd.memset(V5[:, :, :, 1, 0:D], 1.0)

            # ---- emission helpers -------------------------------------------
            def qk_proj_steps(mt):
                """Q & K projection of m-tile mt, as filler closures."""
                steps = []
                for w_sb, b_sb, dst in ((wq_sb, bq_sb, QT_sb), (wk_sb, bk_sb, KT_sb)):
                    for c0, c1 in CHS:
                        def step(w_sb=w_sb, b_sb=b_sb, dst=dst, c0=c0, c1=c1):
                            ps = ps1.tile([P, c1 - c0], F32, tag="ps1")
                            for kt in range(KT):
                                nc.tensor.matmul(
                                    ps[:],
                                    w_sb[:, kt, mt * P : (mt + 1) * P],
                                    x_sb[:, kt, c0:c1],
                                    start=(kt == 0),
                                    stop=(kt == KT - 1),
                                )
                            nc.vector.tensor_scalar_add(
                                dst[:, mt, c0:c1], ps[:], b_sb[:, mt : mt + 1]
                            )
                        steps.append(step)
                return steps

            def v_proj_steps():
                steps = []
                for tt in range(NT):
                    def step(tt=tt):
                        psv = ps1.tile([P, CP], F32, tag="ps1")
                        for kt in range(KT):
                            nc.tensor.matmul(
                                psv[:],
                                x_sb[:, kt, tt * P : (tt + 1) * P],
                                wv_sb[:, kt, :],
                                start=(kt == 0),
                                stop=(kt == KT - 1),
                            )
                        src = psv.rearrange("p (hh par d) -> p par hh d", par=2, d=D)
                        bvr = bv_sb.rearrange("p (hh par d) -> p par hh d", par=2, d=D)
                        nc.vector.tensor_add(V5[:, tt, :, 0, 0:D], src[:, 0], bvr[:, 0])
                        nc.vector.tensor_add(V5[:, tt, :, 1, D:P], src[:, 1], bvr[:, 1])
                    steps.append(step)
                return steps

            def out_proj_steps(s2lo, s2hi, mode):
                """Output projection contribution of head-pairs [s2lo, s2hi).

                mode: "init" psum -> oacc copy; "accum" psum + oacc -> oacc;
                "final" psum + oacc -> bf16 out tile, DMA (whole rows, queues
                round-robined to spread descriptor load).
                """
                dmaq = [nc.sync, nc.scalar, nc.gpsimd]
                steps = []
                for tt in range(NT):
                    ots = [None]
                    for ci, (c0, c1) in enumerate(((0, 512), (512, C))):
                        def step(tt=tt, ci=ci, c0=c0, c1=c1, ots=ots):
                            ps = ps1.tile([P, c1 - c0], F32, tag="ps1")
                            for s2 in range(s2lo, s2hi):
                                nc.tensor.matmul(
                                    ps[:],
                                    y_sb[:, s2, tt * P : (tt + 1) * P],
                                    wp_sb[:, s2, c0:c1],
                                    start=(s2 == s2lo),
                                    stop=(s2 == s2hi - 1),
                                )
                            if mode == "init":
                                nc.vector.tensor_copy(oacc[:, tt, c0:c1], ps[:])
                            elif mode == "accum":
                                nc.vector.tensor_add(
                                    oacc[:, tt, c0:c1], ps[:], oacc[:, tt, c0:c1]
                                )
                            else:
                                if ots[0] is None:
                                    ots[0] = outp.tile(
                                        [P, C], BF16, tag="out", name=f"ot{tt}"
                                    )
                                ot = ots[0]
                                nc.vector.tensor_add(
                                    ot[:, c0:c1], ps[:], oacc[:, tt, c0:c1]
                                )
                                if ci == 1:
                                    dmaq[tt % 3].dma_start(
                                        out[tt * P : (tt + 1) * P, :], ot[:]
                                    )
                        steps.append(step)
                return steps

            oacc = consts.tile([P, NT, C], F32)

            def attn_pair(p, fillers):
                """Attention for head pair (2p, 2p+1); pops filler closures
                between tensor bursts to keep the PE busy while Scalar exp
                and DVE normalize catch up."""
                def fill(n):
                    for _ in range(n):
                        if fillers:
                            fillers.pop(0)()

                hA, hB = 2 * p, 2 * p + 1
                eAs, eBs = [], []
                # head A's att@V rides one kt behind the QK^T/exp stream so the
                # per-kt tensor work matches the Scalar exp pace
                ypsA = [
                    psy.tile([P, c1 - c0], F32, tag="psy", name=f"ypsA{ci}")
                    for ci, (c0, c1) in enumerate(CHS)
                ]
                for kt in range(NT):
                    psA = ps2.tile([P, TT], F32, tag="ps2")
                    psB = ps2.tile([P, TT], F32, tag="ps2")
                    for c0, c1 in CHS:
                        nc.tensor.matmul(
                            psA[:, c0:c1],
                            KT_sb[0:D, p, kt * P : (kt + 1) * P],
                            QT_sb[0:D, p, c0:c1],
                            start=True,
                            stop=True,
                        )
                        nc.tensor.matmul(
                            psB[:, c0:c1],
                            KT_sb[D:P, p, kt * P : (kt + 1) * P],
                            QT_sb[D:P, p, c0:c1],
                            start=True,
                            stop=True,
                        )
                    eA = expp.tile([P, TT], BF16, tag="exp")
                    nc.scalar.activation(
                        eA[:], psA[:], EXP, bias=mk_sb[:, kt : kt + 1], scale=SCALE
                    )
                    eB = expp.tile([P, TT], BF16, tag="exp")
                    nc.scalar.activation(
                        eB[:], psB[:], EXP, bias=mk_sb[:, kt : kt + 1], scale=SCALE
                    )
                    eAs.append(eA)
                    eBs.append(eB)
                    if kt > 0:
                        for ci, (c0, c1) in enumerate(CHS):
                            nc.tensor.matmul(
                                ypsA[ci][:],
                                V_sb[:, kt - 1, hA, :],
                                eAs[kt - 1][:, c0:c1],
                                start=(kt - 1 == 0),
                                stop=False,
                            )
                    fill(1)

                def normalize(h, yps, chunked):
                    yr = slice(0, D) if h % 2 == 0 else slice(D, P)
                    sr = slice(D, P) if h % 2 == 0 else slice(0, D)
                    for ci, (c0, c1) in enumerate(CHS):
                        src = yps[ci] if chunked else yps[0][:, c0:c1]
                        nc.scalar.activation(lnst[sr, c0:c1], src[sr, :], LN)
                    nc.scalar.activation(bpsf[sr, :], lnst[sr, :], EXP, scale=-1.0)
                    for ci, (c0, c1) in enumerate(CHS):
                        src = yps[ci] if chunked else yps[0][:, c0:c1]
                        nc.vector.tensor_mul(
                            y_sb[yr, p, c0:c1], src[yr, :], bpsf[sr, c0:c1]
                        )

                for ci, (c0, c1) in enumerate(CHS):
                    nc.tensor.matmul(
                        ypsA[ci][:],
                        V_sb[:, NT - 1, hA, :],
                        eAs[NT - 1][:, c0:c1],
                        start=(NT == 1),
                        stop=True,
                    )
                normalize(hA, ypsA, chunked=True)
                fill(1)
                # head B accumulates in a ps2-pool tile so its PSUM lifetime
                # doesn't collide with head A's normalize chain
                ypsB = ps2.tile([P, TT], F32, tag="ps2")
                for kt in range(NT):
                    for c0, c1 in CHS:
                        nc.tensor.matmul(
                            ypsB[:, c0:c1],
                            V_sb[:, kt, hB, :],
                            eBs[kt][:, c0:c1],
                            start=(kt == 0),
                            stop=(kt == NT - 1),
                        )
                fill(1)
                normalize(hB, [ypsB], chunked=False)
                fill(2)

            fillers0 = v_proj_steps() + qk_proj_steps(1)
            fillers1 = qk_proj_steps(2)
            fillers2 = qk_proj_steps(3) + out_proj_steps(0, 2, mode="init")
            fillers3 = out_proj_steps(2, 3, mode="accum")

            for step in qk_proj_steps(0):
                step()
            attn_pair(0, fillers0)
            for f in fillers0:
                f()
            attn_pair(1, fillers1)
            for f in fillers1:
                f()
            attn_pair(2, fillers2)
            for f in fillers2:
                f()
            attn_pair(3, fillers3)
            for f in fillers3:
                f()
            for step in out_proj_steps(3, 4, mode="final"):
                step()

    _split_multi_waits(nc)
    return nc


_NC_CACHE = {}


def _get_nc(TT):
    if TT not in _NC_CACHE:
        _NC_CACHE[TT] = _build_nc(TT)
    return _NC_CACHE[TT]


def kernel(x, x_padding_judge, Wq, bq, Wk, bk, Wv, bv, Wp, bp):
    global LAST_RESULTS
    x = np.asarray(x, dtype=np.float32)
    pad = np.asarray(x_padding_judge, dtype=np.float32)
    Wq = np.asarray(Wq, dtype=np.float32)
    Wk = np.asarray(Wk, dtype=np.float32)
    Wv = np.asarray(Wv, dtype=np.float32)
    Wp = np.asarray(Wp, dtype=np.float32)
    bq = np.asarray(bq, dtype=np.float32)
    bk = np.asarray(bk, dtype=np.float32)
    bv = np.asarray(bv, dtype=np.float32)
    bp = np.asarray(bp, dtype=np.float32)
    bf = ml_dtypes.bfloat16

    idxs = [np.nonzero(pad[b] == 0.0)[0] for b in range(B)]
    tns = [len(ix) for ix in idxs]
    TT = max(P, -(-max(tns) // P) * P)

    def pmajor(m, kt):  # [kt*P, n] -> [P, kt*n]
        return np.ascontiguousarray(
            m.reshape(kt, P, -1).transpose(1, 0, 2).reshape(P, -1)
        )

    in_maps = []
    for c in range(8):
        b, s = c // 2, c % 2
        cols = slice(s * CP, (s + 1) * CP)
        xg = np.zeros((TT, C), dtype=np.float32)
        xg[: tns[b]] = x[b][idxs[b]]
        mkv = np.full(TT, -1e9, dtype=np.float32)
        mkv[: tns[b]] = 0.0
        in_maps.append({
            "xh": pmajor(np.ascontiguousarray(xg.T), KT).astype(bf),
            "wq": pmajor(Wq[:, cols], KT).astype(bf),
            "wk": pmajor(Wk[:, cols], KT).astype(bf),
            "wv": pmajor(Wv[:, cols], KT).astype(bf),
            "wp": pmajor(Wp[cols, :], MT).astype(bf),
            "bq": np.ascontiguousarray(bq[cols].reshape(MT, P).T),
            "bk": np.ascontiguousarray(bk[cols].reshape(MT, P).T),
            "bv": np.broadcast_to(bv[cols], (P, CP)).copy(),
            "mk": np.ascontiguousarray(mkv.reshape(TT // P, P).T),
        })

    res = run_bass_kernel_spmd(_get_nc(TT), in_maps, core_ids=list(range(8)))
    LAST_RESULTS = res

    out = np.empty((B, T, C), dtype=np.float32)
    for b in range(B):
        part = res.results[2 * b]["out"].astype(np.float32) + res.results[
            2 * b + 1
        ]["out"].astype(np.float32)
        out[b, idxs[b], :] = part[: tns[b]] + bp

    # fully-padded query rows: uniform attention over ALL keys
    for b in range(B):
        rows = np.nonzero(pad[b] == 1.0)[0]
        if rows.size:
            xbar = x[b].mean(axis=0)
            out[b, rows, :] = (xbar @ Wv + bv) @ Wp + bp
    return out
